# revision 1
# baseline (speedup 1.0000x reference)
"""GIN message-passing encoder (3 layers) on 8 Trainium2 NeuronCores.

Problem: x_{l+1} = relu(BN(relu((x + agg(x)) @ W1 + b1) @ W2 + b2)),
agg[b, d] = sum over edges (s -> d) of x[b, s]; output = stack of the 3
layer outputs, shape [3, 16, 1024, 256].

Strategy
--------
- Data parallel over batch: B=16 split as 2 batch elements per core.
- The scatter-add is a dense matmul against a host-built (N x N) matrix
  Bm[s, d] = I[s, d] + multiplicity(edge s -> d), so
  m0 = (A + I) @ x = Bm^T-contracted matmul; the +x of GIN(eps=0) is the
  identity fold.
- Eval-mode BatchNorm is folded into W2/b2 on the host.
- All matmuls run as float32r (full PE rate at moving-free >= 256).
- Per layer per batch:
    step1: m0T[f, n]  = x-chunks^T (stationary) @ Bm-chunks (moving), PSUM
           -> DVE copy to SBUF (f32r)
    step2: h1T[g, n]  = W1 (stationary) @ m0T (moving)
           -> ACT Relu + per-partition bias b1 straight from PSUM (f32r)
    step3: y[n, gout] = h1T-chunks (stationary) @ W2' (moving)
           -> DVE add of broadcast bias b2', ACT Relu -> next x (f32r)
  x stays in "normal" layout [n, f] which is exactly what step1 consumes
  as stationary chunks, so no transposes anywhere.
"""

import os

import numpy as np

BN_EPS = 1e-5

B, N, F = 16, 1024, 256
L = 3
NCORES = 8
BPC = B // NCORES  # batch elements per core
P = 128
NT = N // P  # 8 node tiles
FT = F // P  # 2 feature tiles
HALF = 512   # moving free-dim chunk
NH = N // HALF  # 2 halves of the node dim

_cache: dict = {}


def _build_nc():
    import concourse.bacc as bacc
    import concourse.mybir as mybir
    import concourse.tile as tile

    F32 = mybir.dt.float32
    F32R = mybir.dt.float32r
    Relu = mybir.ActivationFunctionType.Relu
    Alu = mybir.AluOpType

    nc = bacc.Bacc()

    x0_d = nc.dram_tensor("x0", [BPC, N, F], F32R, kind="ExternalInput")
    bm_d = nc.dram_tensor("bm", [N, N], F32R, kind="ExternalInput")
    w1_d = nc.dram_tensor("w1", [L, F, F], F32R, kind="ExternalInput")
    w2_d = nc.dram_tensor("w2", [L, F, F], F32R, kind="ExternalInput")
    b1_d = nc.dram_tensor("b1", [P, L * FT], F32, kind="ExternalInput")
    b2_d = nc.dram_tensor("b2", [P, L, HALF], F32, kind="ExternalInput")
    out_d = nc.dram_tensor("out", [L, BPC, N, F], F32R, kind="ExternalOutput")

    with tile.TileContext(nc) as tc:
        with (
            tc.tile_pool(name="const", bufs=1) as cpool,
            tc.tile_pool(name="xp", bufs=2) as xpool,
            tc.tile_pool(name="work", bufs=3) as wpool,
            tc.tile_pool(name="yt", bufs=6) as ypool,
            tc.tile_pool(name="pm0", bufs=3, space="PSUM") as pm0,
            tc.tile_pool(name="ph1", bufs=2, space="PSUM") as ph1,
            tc.tile_pool(name="py", bufs=3, space="PSUM") as py,
        ):
            b_sb = cpool.tile([P, NT, N], F32R)
            w1_sb = cpool.tile([P, L, FT, F], F32R)
            w2_sb = cpool.tile([P, L, FT, F], F32R)
            b1_sb = cpool.tile([P, L * FT], F32)
            b2_sb = cpool.tile([P, L, HALF], F32)

            x_cur = xpool.tile([P, BPC, NT, F], F32R, tag="x")

            # Load order matters: per-DMA issue on the Sync queue is
            # ~620 ns, so coalesce chunks and stage the bytes the first
            # step-1 groups need (Bm half 0, batch 0) first.
            for k2 in range(0, NT, 2):
                nc.sync.dma_start(
                    b_sb[:, k2:k2 + 2, 0:HALF],
                    bm_d[k2 * P:(k2 + 2) * P, 0:HALF].rearrange(
                        "(c p) d -> p c d", p=P
                    ),
                )
            for k4 in range(0, NT, 4):
                nc.sync.dma_start(
                    x_cur[:, 0, k4:k4 + 4, :],
                    x0_d[0, k4 * P:(k4 + 4) * P, :].rearrange(
                        "(c p) f -> p c f", p=P
                    ),
                )
            nc.sync.dma_start(
                w1_sb[:, 0], w1_d[0].rearrange("(c p) g -> p c g", p=P)
            )
            nc.sync.dma_start(b1_sb[:], b1_d[:])
            nc.sync.dma_start(b2_sb[:], b2_d[:])
            for k2 in range(0, NT, 2):
                nc.sync.dma_start(
                    b_sb[:, k2:k2 + 2, HALF:N],
                    bm_d[k2 * P:(k2 + 2) * P, HALF:N].rearrange(
                        "(c p) d -> p c d", p=P
                    ),
                )
            for k4 in range(0, NT, 4):
                nc.sync.dma_start(
                    x_cur[:, 1, k4:k4 + 4, :],
                    x0_d[1, k4 * P:(k4 + 4) * P, :].rearrange(
                        "(c p) f -> p c f", p=P
                    ),
                )
            nc.sync.dma_start(
                w2_sb[:, 0], w2_d[0].rearrange("(c p) g -> p c g", p=P)
            )
            for l in range(1, L):
                nc.sync.dma_start(
                    w1_sb[:, l], w1_d[l].rearrange("(c p) g -> p c g", p=P)
                )
                nc.sync.dma_start(
                    w2_sb[:, l], w2_d[l].rearrange("(c p) g -> p c g", p=P)
                )

            for l in range(L):
                x_next = xpool.tile([P, BPC, NT, F], F32R, tag="x")
                for b in range(BPC):
                    # ---- step 1: m0T = (A + I) @ x, transposed layout ----
                    m0t = wpool.tile([P, FT, N], F32R, tag="m0t")
                    for half in range(NH):
                        for ft in range(FT):
                            ps = pm0.tile([P, HALF], F32, tag="pm0")
                            for k in range(NT):
                                nc.tensor.matmul(
                                    ps[:],
                                    x_cur[:, b, k, ft * P:(ft + 1) * P],
                                    b_sb[:, k, half * HALF:(half + 1) * HALF],
                                    start=(k == 0),
                                    stop=(k == NT - 1),
                                )
                            nc.vector.tensor_copy(
                                m0t[:, ft, half * HALF:(half + 1) * HALF], ps[:]
                            )
                    # ---- step 2: h1T = relu(W1^T-contract @ m0T + b1) ----
                    h1t = wpool.tile([P, FT, N], F32R, tag="h1t")
                    for gt in range(FT):
                        for half in range(NH):
                            ps = ph1.tile([P, HALF], F32, tag="ph1")
                            for fk in range(FT):
                                nc.tensor.matmul(
                                    ps[:],
                                    w1_sb[:, l, fk, gt * P:(gt + 1) * P],
                                    m0t[:, fk, half * HALF:(half + 1) * HALF],
                                    start=(fk == 0),
                                    stop=(fk == FT - 1),
                                )
                            nc.scalar.activation(
                                h1t[:, gt, half * HALF:(half + 1) * HALF],
                                ps[:],
                                Relu,
                                bias=b1_sb[:, l * FT + gt:l * FT + gt + 1],
                            )
                    # ---- step 3: y = h1 @ W2' + b2', relu -> next x ----
                    for tp in range(NT // 2):
                        ps = py.tile([P, 2, F], F32, tag="py")
                        for j in range(2):
                            nt = 2 * tp + j
                            for gk in range(FT):
                                nc.tensor.matmul(
                                    ps[:, j, :],
                                    h1t[:, gk, nt * P:(nt + 1) * P],
                                    w2_sb[:, l, gk, :],
                                    start=(gk == 0),
                                    stop=(gk == FT - 1),
                                )
                        ytmp = ypool.tile([P, 2, F], F32, tag="ytmp")
                        nc.vector.scalar_tensor_tensor(
                            ytmp[:],
                            ps[:],
                            1.0,
                            b2_sb[:, l, :].rearrange("p (a f) -> p a f", a=2),
                            op0=Alu.mult,
                            op1=Alu.add,
                        )
                        nc.scalar.activation(
                            x_next[:, b, 2 * tp:2 * tp + 2, :], ytmp[:], Relu
                        )
                        nc.sync.dma_start(
                            out_d[l, b, 2 * tp * P:(2 * tp + 2) * P, :].rearrange(
                                "(t p) f -> p t f", p=P
                            ),
                            x_next[:, b, 2 * tp:2 * tp + 2, :],
                        )
                x_cur = x_next

    nc.finalize()
    return nc


def kernel(h, edge_index, W1, b1, W2, b2, gamma, beta, run_mean, run_var):
    from concourse.bass_utils import run_bass_kernel_spmd

    h = np.asarray(h, dtype=np.float32)
    edge_index = np.asarray(edge_index)
    W1 = np.asarray(W1, dtype=np.float32)
    b1 = np.asarray(b1, dtype=np.float32)
    W2 = np.asarray(W2, dtype=np.float32)
    b2 = np.asarray(b2, dtype=np.float32)
    gamma = np.asarray(gamma, dtype=np.float32)
    beta = np.asarray(beta, dtype=np.float32)
    run_mean = np.asarray(run_mean, dtype=np.float32)
    run_var = np.asarray(run_var, dtype=np.float32)

    # host-side preprocessing
    src = edge_index[0].astype(np.int64)
    dst = edge_index[1].astype(np.int64)
    bm = np.zeros((N, N), dtype=np.float32)
    np.add.at(bm, (src, dst), 1.0)
    bm[np.arange(N), np.arange(N)] += 1.0

    inv = (gamma / np.sqrt(run_var + BN_EPS)).astype(np.float32)      # [L, F]
    w2f = (W2 * inv[:, None, :]).astype(np.float32)                   # [L, F, F]
    b2f = (b2 * inv + beta - run_mean * inv).astype(np.float32)       # [L, F]

    # b1 as per-partition scalars: [P, L*FT], column l*FT+gt = b1[l, gt*128:...]
    b1r = np.ascontiguousarray(
        b1.reshape(L, FT, P).transpose(2, 0, 1).reshape(P, L * FT)
    )
    # b2' broadcast across partitions, twice along free (for [P, 2, F] pairs)
    b2r = np.ascontiguousarray(
        np.broadcast_to(
            np.concatenate([b2f, b2f], axis=1)[None], (P, L, HALF)
        )
    )

    if "nc" not in _cache:
        _cache["nc"] = _build_nc()
    nc = _cache["nc"]

    in_maps = []
    for c in range(NCORES):
        in_maps.append({
            "x0": np.ascontiguousarray(h[c * BPC:(c + 1) * BPC]),
            "bm": bm,
            "w1": W1,
            "w2": w2f,
            "b1": b1r,
            "b2": b2r,
        })

    trace = os.environ.get("KERNEL_TRACE") == "1"
    res = run_bass_kernel_spmd(
        nc, in_maps, core_ids=list(range(NCORES)), trace=trace
    )
    _cache["last_results"] = res
    return np.concatenate([r["out"] for r in res.results], axis=1)

